# revision 10
# baseline (speedup 1.0000x reference)
"""Trainium2 Bass kernel for nn_Attention (B=2, S=2048, D=512, H=8).

Sharding: 8 cores = 2 batches x 4 head-groups (2 heads each).

Algebraic fusions (exact, host-side weight preprocessing in f64):
  W_full = W_multi @ W_sep, G_h = Wq_h^T Wk_h (k-projection vanishes),
  Wvp_h = (Wres_h @ Wv_h)^T (restore matmul vanishes).

Precision scheme: the two big matmuls (S = QK^T and PV) run fp8e4 +
DoubleRow (0.5 cy/row, 4x bf16); qtT runs fp8-DR with host-side residual
splits; V' runs one bf16 pass. All fp8 tensors are power-of-2 prescaled
into e4m3's normal range (min normal 2^-6 -- G/Wvp at ~0.004-0.009 rms
would otherwise quantize as subnormals):
  x8/xr = split8(8x), G8/Gr = split8(16G)  -> scores carry 1024, folded
  into the exp scale. Wbf = bf16(256 Wvp) -> V8 carries 256.
Softmax via shifted weights: E = exp(s/sqrt(D)) = 1 + e; only e (small,
~0.2) is quantized to fp8, so weight noise is ~5x smaller than fp8(E):
  num = c_h + sum_k e8_k V8_k,   c_h = 256 * sum_k F_k V'_k   (host f64)
  den = den_c + sum_k e8_k F8_k, den_c = 128 * sum_k F_k      (host f64)
with F = exp(k-side bias) == 1 for zero biases; F8 = fp8(128 F) = 128
exactly. Corrections enter as rank-2 bf16 matmuls that initialize each
PSUM accumulation group. num/den scale ratio (2) divides out on host.
V-side bias and b_res add on host (sum_k w_k = 1 makes this exact).
"""

import numpy as np

P = 128
B = 2
S = 2048
D = 512          # word dim == head dim
H = 8            # total heads
E3 = 3 * D
NHL = 2          # local heads per core
NC = 8           # cores
CH = 512         # sq chunk width
NCH = S // CH    # 4
NT = S // P      # 16 sk tiles
KD = D // P      # 4 contraction k-tiles
INV_SQRT_D = 1.0 / float(np.sqrt(np.float32(D)))
SX = 8.0         # x fp8 prescale
SG = 16.0        # G prescale (scores carry SG*SX^2 = 1024)
S_SCORE = SG * SX * SX
NUMS = 128.0     # V8 / c scale (V' max ~1.01 -> 129 < fp8e4 max 240;
                 # the device fp8 cast does NOT saturate, it makes Inf)
DENS = 128.0     # F8 / den_c scale

_CACHE = {}


def _build_nc(zero_bias: bool):
    import concourse.mybir as mybir
    import concourse.tile as tile
    from concourse import bacc

    dt = mybir.dt
    BF = dt.bfloat16
    F32 = dt.float32
    FP8 = dt.float8e4
    Act = mybir.ActivationFunctionType
    Alu = mybir.AluOpType
    DR = mybir.MatmulPerfMode.DoubleRow

    nc = bacc.Bacc("TRN2", target_bir_lowering=False, debug=False, num_devices=NC)

    x8_d = nc.declare_dram_parameter("x8", [D, S], FP8, isOutput=False)
    xr_d = nc.declare_dram_parameter("xr", [D, S], FP8, isOutput=False)
    xb_d = nc.declare_dram_parameter("xb", [D, S], BF, isOutput=False)
    g8_d = nc.declare_dram_parameter("G8", [NHL, D, D], FP8, isOutput=False)
    gr_d = nc.declare_dram_parameter("Gr", [NHL, D, D], FP8, isOutput=False)
    wb_d = nc.declare_dram_parameter("Wb", [NHL, D, D], BF, isOutput=False)
    f8_d = nc.declare_dram_parameter("F8", [NHL, S], FP8, isOutput=False)
    cp_d = nc.declare_dram_parameter("cp", [NHL, 2, D], BF, isOutput=False)
    dc_d = nc.declare_dram_parameter("dc", [NHL, 2, 1], BF, isOutput=False)
    if not zero_bias:
        fs_d = nc.declare_dram_parameter("Fs", [NHL, S], F32, isOutput=False)
    out_d = nc.declare_dram_parameter("out", [S, D], F32, isOutput=True)
    import os
    KDEBUG = os.environ.get("KDEBUG", "0") == "1"
    if KDEBUG:
        q8o_d = [nc.declare_dram_parameter(f"q8o{h}", [P, KD * S], FP8, isOutput=True) for h in range(NHL)]
        v8o_d = [nc.declare_dram_parameter(f"v8o{h}", [P, NT * D], FP8, isOutput=True) for h in range(NHL)]
        e8o_d = [nc.declare_dram_parameter(f"e8o{h}", [P, NT * CH], FP8, isOutput=True) for h in range(NHL)]

    with tile.TileContext(nc) as tc:
        with (
            tc.tile_pool(name="w", bufs=1) as wp,
            tc.tile_pool(name="psum", bufs=1, space="PSUM") as pp,
        ):
            ones2 = wp.tile([2, P], BF, tag="ones2")
            nc.vector.memset(ones2[:], 1.0)
            bias0 = wp.tile([P, 1], F32, tag="bias0")
            nc.vector.memset(bias0[:], 0.0)

            x8 = wp.tile([P, KD * S], FP8, tag="x8", name="x8")
            xr = wp.tile([P, KD * S], FP8, tag="xr", name="xr")
            xb = wp.tile([P, KD * S], BF, tag="xb", name="xb")
            g8s, grs, wbs, f8s, cps, dcs, fss = [], [], [], [], [], [], []
            for h in range(NHL):
                g8s.append(wp.tile([P, KD * D], FP8, tag=f"G8{h}", name=f"G8{h}"))
                grs.append(wp.tile([P, KD * D], FP8, tag=f"Gr{h}", name=f"Gr{h}"))
                wbs.append(wp.tile([P, KD * D], BF, tag=f"Wb{h}", name=f"Wb{h}"))
                f8s.append(wp.tile([P, NT], FP8, tag=f"F8{h}", name=f"F8{h}"))
                cps.append(wp.tile([2, D], BF, tag=f"cp{h}", name=f"cp{h}"))
                dcs.append(wp.tile([2, 1], BF, tag=f"dc{h}", name=f"dc{h}"))
                if not zero_bias:
                    fss.append(wp.tile([P, NT], F32, tag=f"Fs{h}", name=f"Fs{h}"))

            x8_v = x8[:].rearrange("p (k s) -> p k s", k=KD)
            xr_v = xr[:].rearrange("p (k s) -> p k s", k=KD)
            xb_v = xb[:].rearrange("p (k s) -> p k s", k=KD)
            g8_v = [g8s[h][:].rearrange("p (k d) -> p k d", k=KD) for h in range(NHL)]
            gr_v = [grs[h][:].rearrange("p (k d) -> p k d", k=KD) for h in range(NHL)]
            wb_v = [wbs[h][:].rearrange("p (k d) -> p k d", k=KD) for h in range(NHL)]
            f8_v = [f8s[h][:].rearrange("p (t u) -> p t u", u=1) for h in range(NHL)]

            x8d_v = x8_d[:].rearrange("(k p) s -> p k s", p=P)
            xrd_v = xr_d[:].rearrange("(k p) s -> p k s", p=P)
            xbd_v = xb_d[:].rearrange("(k p) s -> p k s", p=P)

            # DMA order: first-needed first, split across sync/gpsimd queues.
            nc.sync.dma_start(g8_v[0], g8_d[0, :, :].rearrange("(k p) d -> p k d", p=P))
            nc.gpsimd.dma_start(x8_v[:, :, 0 : CH // 2], x8d_v[:, :, 0 : CH // 2])
            nc.sync.dma_start(gr_v[0], gr_d[0, :, :].rearrange("(k p) d -> p k d", p=P))
            nc.gpsimd.dma_start(x8_v[:, :, CH // 2 : CH], x8d_v[:, :, CH // 2 : CH])
            for s in range(1, NCH):
                nc.gpsimd.dma_start(
                    x8_v[:, :, s * CH : (s + 1) * CH], x8d_v[:, :, s * CH : (s + 1) * CH]
                )
            for s in range(NCH):
                nc.gpsimd.dma_start(
                    xr_v[:, :, s * CH : (s + 1) * CH], xrd_v[:, :, s * CH : (s + 1) * CH]
                )
            nc.sync.dma_start(wb_v[0], wb_d[0, :, :].rearrange("(k p) d -> p k d", p=P))
            for s in range(NCH):
                nc.sync.dma_start(
                    xb_v[:, :, s * CH : (s + 1) * CH], xbd_v[:, :, s * CH : (s + 1) * CH]
                )
            nc.sync.dma_start(g8_v[1], g8_d[1, :, :].rearrange("(k p) d -> p k d", p=P))
            nc.sync.dma_start(gr_v[1], gr_d[1, :, :].rearrange("(k p) d -> p k d", p=P))
            nc.sync.dma_start(wb_v[1], wb_d[1, :, :].rearrange("(k p) d -> p k d", p=P))
            for h in range(NHL):
                nc.sync.dma_start(f8s[h][:], f8_d[h, :].rearrange("(t p) -> p t", p=P))
                nc.sync.dma_start(cps[h][:], cp_d[h, :, :])
                nc.sync.dma_start(dcs[h][:], dc_d[h, :, :])
                if not zero_bias:
                    nc.sync.dma_start(
                        fss[h][:], fs_d[h, :].rearrange("(t p) -> p t", p=P)
                    )

            out_acc = [
                wp.tile([P, D], F32, tag=f"oacc{st}", name=f"oacc{st}")
                for st in range(NT)
            ]

            for h in range(NHL):
                # ---- qtT = G^T x^T, 3 fp8-DR passes, f32 psum -> q8 fp8 ----
                q8 = wp.tile([P, KD * S], FP8, tag="q8", bufs=2, name=f"q8_{h}")
                q8_v = q8[:].rearrange("p (k s) -> p k s", k=KD)
                for s in range(NCH):
                    for mp in range(2):
                        acc = pp.tile([P, 2 * CH], F32, tag="wide", bufs=2)
                        for half in range(2):
                            m = 2 * mp + half
                            ot = acc[:, half * CH : (half + 1) * CH]
                            first = True
                            for ga, xa in ((g8_v, x8_v), (gr_v, x8_v), (g8_v, xr_v)):
                                for kp in range(2):
                                    nc.tensor.matmul(
                                        ot,
                                        ga[h][:, 2 * kp : 2 * kp + 2, m * P : (m + 1) * P],
                                        xa[:, 2 * kp : 2 * kp + 2, s * CH : (s + 1) * CH],
                                        start=first,
                                        stop=(ga is g8_v and xa is xr_v and kp == 1),
                                        perf_mode=DR,
                                    )
                                    first = False
                        nc.scalar.activation(
                            q8_v[:, 2 * mp : 2 * mp + 2, s * CH : (s + 1) * CH],
                            acc[:].rearrange("p (u c) -> p u c", u=2),
                            Act.Copy,
                        )

                # ---- V' = x Wvp, one bf16 pass -> V8 fp8 -------------------
                v8 = wp.tile([P, NT * D], FP8, tag="v8", bufs=2, name=f"v8_{h}")
                v8_v = v8[:].rearrange("p (t d) -> p t d", t=NT)
                for tp in range(NT // 2):
                    acc = pp.tile([P, 2 * CH], F32, tag="wide", bufs=2)
                    for half in range(2):
                        t = 2 * tp + half
                        ot = acc[:, half * CH : (half + 1) * CH]
                        for kd in range(KD):
                            nc.tensor.matmul(
                                ot,
                                xb[:, kd * S + t * P : kd * S + (t + 1) * P],
                                wbs[h][:, kd * D : (kd + 1) * D],
                                start=(kd == 0),
                                stop=(kd == KD - 1),
                            )
                    if zero_bias:
                        nc.vector.tensor_copy(
                            v8_v[:, 2 * tp : 2 * tp + 2, :],
                            acc[:].rearrange("p (u c) -> p u c", u=2),
                        )
                    else:
                        for half in range(2):
                            t = 2 * tp + half
                            nc.scalar.activation(
                                v8_v[:, 2 * tp + half : 2 * tp + half + 1, :],
                                acc[:, half * CH : (half + 1) * CH].rearrange(
                                    "p (u c) -> p u c", u=1
                                ),
                                Act.Copy,
                                scale=fss[h][:, t : t + 1],
                            )

                # ---- attention, software-pipelined over sq chunks ----------
                e8cs = {}

                def emit_sgroup(c, tp, h=h, q8_v=q8_v):
                    e8_v = e8cs[c]
                    sacc = pp.tile([P, 2 * CH], F32, tag="wide", bufs=2)
                    for half in range(2):
                        t = 2 * tp + half
                        ot = sacc[:, half * CH : (half + 1) * CH]
                        for kp in range(2):
                            nc.tensor.matmul(
                                ot,
                                x8_v[:, 2 * kp : 2 * kp + 2, t * P : (t + 1) * P],
                                q8_v[:, 2 * kp : 2 * kp + 2, c * CH : (c + 1) * CH],
                                start=(kp == 0),
                                stop=(kp == 1),
                                perf_mode=DR,
                            )
                    ebf = wp.tile([P, 2 * CH], F32, tag="ebf", bufs=4)
                    nc.scalar.activation(
                        ebf[:], sacc[:], Act.Exp, bias=bias0[:, 0:1],
                        scale=INV_SQRT_D / S_SCORE,
                    )
                    nc.vector.tensor_scalar_add(
                        e8_v[:, 2 * tp : 2 * tp + 2, :],
                        ebf[:].rearrange("p (u c) -> p u c", u=2),
                        -1.0,
                    )

                def emit_pv(c, j, h=h, v8_v=v8_v):
                    e8_v = e8cs[c]
                    st = c * (CH // P) + j
                    pv = pp.tile([P, D], F32, tag="pv", bufs=2)
                    den = pp.tile([P, 1], F32, tag="den", bufs=2)
                    nc.tensor.matmul(pv[:], ones2[:], cps[h][:], start=True, stop=False)
                    nc.tensor.matmul(den[:], ones2[:], dcs[h][:], start=True, stop=False)
                    for tp in range(NT // 2):
                        elhs = e8_v[:, 2 * tp : 2 * tp + 2, j * P : (j + 1) * P]
                        nc.tensor.matmul(
                            pv[:], elhs, v8_v[:, 2 * tp : 2 * tp + 2, :],
                            start=False, stop=(tp == NT // 2 - 1), perf_mode=DR,
                        )
                        nc.tensor.matmul(
                            den[:], elhs, f8_v[h][:, 2 * tp : 2 * tp + 2, :],
                            start=False, stop=(tp == NT // 2 - 1), perf_mode=DR,
                        )
                    invd = wp.tile([P, 1], F32, tag="invd", bufs=4)
                    nc.vector.reciprocal(invd[:], den[:])
                    if h == 0:
                        nc.vector.tensor_scalar_mul(out_acc[st][:], pv[:], invd[:])
                    else:
                        osb = wp.tile([P, D], F32, tag="osb", bufs=3)
                        nc.vector.scalar_tensor_tensor(
                            osb[:], pv[:], invd[:], out_acc[st][:],
                            Alu.mult, Alu.add,
                        )
                        nc.sync.dma_start(out_d[st * P : (st + 1) * P, :], osb[:])

                import os
                PIPE = os.environ.get("KPIPE", "1") == "1"
                if KDEBUG:
                    nc.sync.dma_start(q8o_d[h][:], q8[:])
                    nc.sync.dma_start(v8o_d[h][:], v8[:])
                prev = None
                for c in range(NCH):
                    e8c = wp.tile(
                        [P, NT * CH], FP8, tag="e8c", bufs=2, name=f"e8_{h}_{c}"
                    )
                    e8cs[c] = e8c[:].rearrange("p (t s) -> p t s", t=NT)
                    for tp in range(NT // 2):
                        emit_sgroup(c, tp)
                        if PIPE and prev is not None and tp % 2 == 1:
                            emit_pv(prev, tp // 2)
                    if KDEBUG and c == 0:
                        nc.sync.dma_start(e8o_d[h][:], e8c[:])
                    if not PIPE:
                        for j in range(CH // P):
                            emit_pv(c, j)
                    prev = c
                if PIPE:
                    for j in range(CH // P):
                        emit_pv(prev, j)

    nc.compile()
    return nc


def _get_nc(zero_bias: bool = True):
    key = ("nc", zero_bias)
    if key not in _CACHE:
        _CACHE[key] = _build_nc(zero_bias)
    return _CACHE[key]


def _prep_inputs(x, W_sep, b_sep, W_multi, b_multi, W_res, b_res):
    """Host-side exact weight fusion (f64) + fp8 residual splits + sharding."""
    import ml_dtypes
    import concourse.mybir as mybir

    bf16 = ml_dtypes.bfloat16
    fp8 = mybir.dt.np(mybir.dt.float8e4)

    x = np.asarray(x, dtype=np.float64)
    W_sep = np.asarray(W_sep, dtype=np.float64)
    b_sep = np.asarray(b_sep, dtype=np.float64)
    W_multi = np.asarray(W_multi, dtype=np.float64)
    b_multi = np.asarray(b_multi, dtype=np.float64)
    W_res = np.asarray(W_res, dtype=np.float64)

    zero_bias = not (np.any(b_sep) or np.any(b_multi))

    W_full = W_multi @ W_sep            # [3*D*H, D]
    b_full = W_multi @ b_sep + b_multi  # [3*D*H]
    Wq = W_full.reshape(H, E3, D)[:, 0:D, :]
    Wk = W_full.reshape(H, E3, D)[:, D : 2 * D, :]
    Wv = W_full.reshape(H, E3, D)[:, 2 * D :, :]
    bq = b_full.reshape(H, E3)[:, 0:D]
    bv = b_full.reshape(H, E3)[:, 2 * D :]
    Wres_h = W_res.reshape(D, H, D).transpose(1, 0, 2)   # [H, dd, d]

    G = np.einsum("hdi,hdj->hij", Wq, Wk)                # [H, Din, Din]
    WvpT = np.einsum("hvi,hdv->hid", Wv, Wres_h)         # [H, Din, Ddd]
    gvec = np.einsum("hdi,hd->hi", Wk, bq) * INV_SQRT_D  # [H, Din]
    bfv = np.einsum("hdv,hv->hd", Wres_h, bv)            # [H, Ddd]

    def split8(a):
        a8 = a.astype(fp8)
        ar = (a - a8.astype(np.float64)).astype(fp8)
        return np.ascontiguousarray(a8), np.ascontiguousarray(ar)

    xT = x.transpose(0, 2, 1)                            # [B, D, S]
    x8, xr = split8(xT * SX)
    xbf = np.ascontiguousarray(xT.astype(bf16))
    G8, Gr = split8(G * SG)
    Wbf = np.ascontiguousarray((WvpT * NUMS).astype(bf16))

    t3 = np.einsum("bsd,hd->bhs", x, gvec)               # [B, H, S]
    F = np.exp(t3)
    F8 = np.ascontiguousarray((F * DENS).astype(fp8))
    den_c = F.sum(axis=2) * DENS                         # [B, H]
    # c[b,h,d] = NUMS * sum_s F[b,h,s] * (x[b] @ WvpT[h])[s,d]  (no v-bias)
    Fx = np.einsum("bhs,bsd->bhd", F, x)                 # [B, H, Din]
    c = np.einsum("bhd,hdi->bhi", Fx, WvpT) * NUMS       # [B, H, Ddd]

    def bfpair(a):
        a1 = a.astype(bf16)
        a2 = (a - a1.astype(np.float64)).astype(bf16)
        return np.stack([a1, a2], axis=-2)               # [..., 2, D]

    cp = bfpair(c)                                       # [B, H, 2, D]
    dc = bfpair(den_c[..., None])                        # [B, H, 2, 1]

    host_bias = bfv.sum(axis=0) + np.asarray(b_res, dtype=np.float64)  # [D]

    in_maps = []
    for core in range(NC):
        b, hg = divmod(core, 4)
        sl = slice(2 * hg, 2 * hg + 2)
        m = {
            "x8": x8[b],
            "xr": xr[b],
            "xb": xbf[b],
            "G8": G8[sl],
            "Gr": Gr[sl],
            "Wb": Wbf[sl],
            "F8": np.ascontiguousarray(F8[b, sl]),
            "cp": np.ascontiguousarray(cp[b, sl]),
            "dc": np.ascontiguousarray(dc[b, sl]),
        }
        if not zero_bias:
            m["Fs"] = np.ascontiguousarray(F[b, sl].astype(np.float32))
        in_maps.append(m)
    return in_maps, host_bias, zero_bias


def kernel(x, W_sep, b_sep, W_multi, b_multi, W_res, b_res):
    from concourse.bass_utils import run_bass_kernel_spmd

    in_maps, host_bias, zero_bias = _prep_inputs(
        x, W_sep, b_sep, W_multi, b_multi, W_res, b_res
    )
    nc = _get_nc(zero_bias)
    res = run_bass_kernel_spmd(nc, in_maps, list(range(NC)), trace=False)

    out = np.zeros((B, S, D), dtype=np.float64)
    for core in range(NC):
        out[core // 4] += np.asarray(res.results[core]["out"], dtype=np.float64)
    out *= DENS / NUMS
    out += host_bias
    return out.astype(np.float32)
